# revision 30
# baseline (speedup 1.0000x reference)
"""MMD loss kernel for Trainium2 (8 NeuronCores, Bass/Tile).

reference math:
  src = X[:2048], tgt = X[2048:],  D=512
  xx = mean over [4096,4096] of sum_k exp(-d2_dup(src,src)/(bw_xx*2^k))
  (dup matrix mean == mean over the 2048^2 block), similarly yy, and
  xy uses the full 4096^2 matrix of X.
  bw for (a,b) = sum(d2([a;b]))/(m^2-m) / mul^(num//2),  mul=2, num=5.

Strategy:
  - bandwidth sums have a closed form: sum_block d2 = 2n*sum(sq) - 2|sum x|^2
    -> computed host-side in fp64, passed to the device as runtime
    activation *scales* (per-partition AP), so no first pass over d2.
  - pairwise tile: PSUM M = G - sq_i/2 - sq_j/2 = -d2/2 via an augmented
    matmul: K=512 data in single-pass bf16 + K=4 aug rows with bf16 hi/lo
    split of -sq/2 (kept high-precision so only the x.y cross term is
    bf16-rounded; full-pipeline rel err ~1.2e-4).
  - 5-kernel sum: u = exp(scale*M) with scale = 1/(8*bw_base) on ACT
    (accum_out rider = sum u), then ONE fused custom-DVE op computes
    u^2+u^4+u^8+u^16 elementwise (8 ALU stages) with accum_out rider =
    its row sum. The host only needs the total of the 5 kernel sums, so
    two riders per chain suffice.
  - symmetry: the distance matrix is symmetric. Own-half coverage is
    minimal: each core computes its diagonal block (w1), one adjacent
    pair block (w2), and HALF (256 cols) of a skip pair (w2) — the two
    halves of each skip pair are computed from the SAME lhsT side by two
    cores (both hold those rows in a dedicated slotL), so the union tiles
    the pair block exactly. Cross src/tgt blocks are covered once with
    weight 2 across the 8 cores. Every core runs the SAME program on a
    per-core permuted column layout (4.5 block-equivalents per core, the
    SPMD minimum for this decomposition; was 5 with the old 1,2,1 cyclic
    scheme).
  - timing contract: only the final rep's riders are DMA'd out, so the
    output size is independent of REPEAT and the wall-delta between
    REPEAT variants isolates on-device body time.
"""

import sys

sys.path.insert(0, "/opt/trn_rl_repo")

import numpy as np
import ml_dtypes

N, D, HALF, BLK = 4096, 512, 2048, 512
NCORES = 8
NSTRIP = 4          # 4 strips of 128 rows per core
RID_W = 2           # rider slots per unit: [sum u, sum u^2+u^4+u^8+u^16]

# Local column layout (units of X-row indices), per core:
#   own   [0:512)      core's own row group g = 4*half + k0
#   adj   [512:1024)   group (k0+1)%4 of the same half      -> pair w2
#   slotL [1024:1536)  lhsT rows for the skip pair: group k0%2
#   slotR [1536:1792)  rhs cols for the skip pair: 256-col half of
#                      group (k0%2)+2  (first half if k0<2 else second)
#   cross [1792:2816)  two 512-col groups of the other half -> w2, xy only
# Own-half coverage proof: diag (g,g) w1 by each core; adjacent pairs
# (k,k+1 mod 4) w2 once each; skip pairs (0,2),(1,3) w2 split into two
# 256-col halves, both computed with the SAME lhsT side (cores k and k+2
# hold those rows in slotL), so the union tiles the pair block exactly.
LC = 2816
AUG_W = LC + 1024   # + [1,1,hi,lo] lhsT sections for own and slotL
OWN_OFF, ADJ_OFF, SL_OFF, SR_OFF = 0, 512, 1024, 1536
CR0_OFF, CR1_OFF = 1792, 2304

# units: (rhs_off, width, lhsT_off, aug_lhsT_off, weight, chain)
# the short skip-pair instance goes LAST: in 3rd position its small matmul
# burst starves the PSUM double-buffer pipeline (sim: 21.3us vs 19.2us/rep)
def _units():
    a_own = LC
    a_sl = LC + 512
    us = []
    for off, w, l, al, wt in (
        (OWN_OFF, 512, OWN_OFF, a_own, 1),   # diag block
        (ADJ_OFF, 512, OWN_OFF, a_own, 2),   # adjacent pair
    ):
        us.append((off, w, l, al, wt, "own"))
        us.append((off, w, l, al, wt, "xy"))
    us.append((CR0_OFF, 512, OWN_OFF, a_own, 2, "xy"))
    us.append((CR1_OFF, 512, OWN_OFF, a_own, 2, "xy"))
    us.append((SR_OFF, 256, SL_OFF, a_sl, 2, "own"))   # skip-pair half
    us.append((SR_OFF, 256, SL_OFF, a_sl, 2, "xy"))
    return us


UNITS = _units()
NUNIT = len(UNITS)  # 8

# NOTE: fp8-e4m3 DoubleRow Gram matmuls (K=256/instruction) were tried and
# measured SLOWER on real HW than plain bf16 (26.1us vs 24.1us body) despite
# the cost model predicting a win — DR LDWEIGHTS overhead isn't hidden here.
MM_DT = "bfloat16"
U_DT = "float32"    # dtype of the exp output / fused-op scratch tiles


REPEAT = 1


_MMD_OP = None


def _get_mmd_op():
    """Fused DVE op: out = u^2+u^4+u^8+u^16, accum_out = row-sum(out).

    Registered once into dve_ops.OPS (the sanctioned custom-DVE extension
    point; the uop table is emitted per-NEFF at compile time)."""
    global _MMD_OP
    if _MMD_OP is not None:
        return _MMD_OP
    from concourse import dve_ops
    from concourse.dve_spec import Spec, Src0, sq, lower
    from concourse.dve_uop import AluOp, DveOpSpec

    name = "MMD_POW_SUM"
    for op in dve_ops.OPS:
        if op.name == name:
            _MMD_OP = op
            return op

    a = sq(Src0)
    b = sq(a)
    c = sq(b)
    d = sq(c)

    def _ref(in0, in1, c0, c1, c2):
        x = in0.astype(np.float32)
        aa = x * x
        bb = aa * aa
        cc = bb * bb
        dd = cc * cc
        body = (aa + bb) + (cc + dd)
        return body, body.reshape(body.shape[0], -1).sum(
            axis=-1, keepdims=True)

    spec = Spec(body=(a + b) + (c + d), accum=AluOp.ADD, reference=_ref)
    row = max(dve_ops._SUB_OPCODE_FOR_NAME.values()) + 1
    assert row < 0x20, "custom-DVE opcode rows exhausted"
    shas = {}
    for ver in ("v3", "v4"):
        uops = lower(spec, ver=ver)
        shas[ver] = DveOpSpec(name=name, opcode=row, uops=uops,
                              rd1_en=False).sha(ver)
    op = dve_ops.DveOp(name, spec, subdim=False, uops_sha=shas)
    dve_ops.OPS.append(op)
    dve_ops._SUB_OPCODE_FOR_NAME[name] = row
    dve_ops.CUSTOM_DVE_SPECS[name] = spec
    _MMD_OP = op
    return op


def _local_cols(core):
    half, k = core // 4, core % 4
    own_base, other_base = half * HALF, (1 - half) * HALF
    own = own_base + 512 * k + np.arange(512)
    adj = own_base + 512 * ((k + 1) % 4) + np.arange(512)
    sl = own_base + 512 * (k % 2) + np.arange(512)
    sr_g = (k % 2) + 2
    sr_base = own_base + 512 * sr_g + (0 if k < 2 else 256)
    sr = sr_base + np.arange(256)
    if half == 0:
        cross = [0, 1] if k % 2 == 0 else [2, 3]
    else:
        cross = [1, 3] if k < 2 else [0, 2]
    cr = [other_base + 512 * b + np.arange(512) for b in cross]
    return np.concatenate([own, adj, sl, sr] + cr)


def _build_program():
    import concourse.bacc as bacc
    import concourse.mybir as mybir
    import concourse.tile as tile

    f32 = mybir.dt.float32
    mm_dt = getattr(mybir.dt, MM_DT)
    u_dt = getattr(mybir.dt, U_DT)
    mmd_op = _get_mmd_op()

    nc = bacc.Bacc("TRN2", target_bir_lowering=False, debug=False,
                   num_devices=NCORES)
    xth_d = nc.dram_tensor("xth", [4, 128, LC], mm_dt, kind="ExternalInput")
    aug_d = nc.dram_tensor("aug", [4, AUG_W], mm_dt, kind="ExternalInput")
    sc_d = nc.dram_tensor("scales", [128, 2], f32, kind="ExternalInput")
    nrep = globals().get("REPEAT", 1)
    # Output only the final rep's riders: keeps the output tensor size
    # independent of REPEAT so the wall-delta timing isolates device time
    # (otherwise the tunnel transfer of the extra output dominates).
    rid_d = nc.dram_tensor("riders", [NUNIT, 128, RID_W], f32,
                           kind="ExternalOutput")

    # instances: (rhs_off, width, lhsT_off, aug_lhsT_off, [unit ids])
    inst = {}
    for u, (off, w, l, al, wt, chain) in enumerate(UNITS):
        inst.setdefault((off, w, l, al), []).append(u)

    with tile.TileContext(nc) as tc:
        with (
            tc.tile_pool(name="xtp", bufs=1) as xtp,
            tc.tile_pool(name="augp", bufs=1) as augp,
            tc.tile_pool(name="scp", bufs=1) as scp,
            tc.tile_pool(name="ridp", bufs=1) as ridp,
            tc.tile_pool(name="psp", bufs=8, space="PSUM") as psp,
            tc.tile_pool(name="up", bufs=4) as up,
        ):
            xth = [xtp.tile([128, LC], mm_dt, tag=f"xth{k}",
                            name=f"xth{k}") for k in range(4)]
            for k in range(4):
                nc.sync.dma_start(out=xth[k][:], in_=xth_d.ap()[k])
            aug = augp.tile([4, AUG_W], mm_dt, tag="aug", name="aug")
            sc = scp.tile([128, 2], f32, tag="sc", name="sc")
            nc.sync.dma_start(out=aug[:], in_=aug_d.ap())
            nc.sync.dma_start(out=sc[:], in_=sc_d.ap())

            riders = [[ridp.tile([128, RID_W], f32, tag=f"rid{u}_{rp}",
                                 name=f"rid{u}_{rp}") for u in range(NUNIT)]
                      for rp in range(nrep)]

            for rep in range(nrep):
                for (off, w, l, al), us in inst.items():
                    nfree = 4 * w  # exp/chain width (4 strips of w)
                    ps = psp.tile([128, 2048], f32, tag="ps", name="ps",
                                  bufs=2)
                    for s in range(4):
                        pss = ps[:, w * s:w * s + w]
                        for k in range(4):
                            lh = xth[k][:, l + 128 * s:l + 128 * s + 128]
                            rh = xth[k][:, off:off + w]
                            nc.tensor.matmul(out=pss, lhsT=lh, rhs=rh,
                                             start=(k == 0), stop=False)
                        nc.tensor.matmul(
                            out=pss,
                            lhsT=aug[:, al + 128 * s:al + 128 * s + 128],
                            rhs=aug[:, off:off + w],
                            start=False, stop=True)

                    for u in us:
                        chain = UNITS[u][5]
                        rid = riders[rep][u]
                        sci = 0 if chain == "own" else 1
                        cur = up.tile([128, 2048], u_dt, tag="u", name="u",
                                      bufs=3)
                        nc.scalar.activation(
                            out=cur[:, 0:nfree], in_=ps[:, 0:nfree],
                            func=mybir.ActivationFunctionType.Exp,
                            scale=sc[:, sci:sci + 1],
                            accum_out=rid[:, 0:1])
                        scr = up.tile([128, 2048], u_dt, tag="usq",
                                      name="usq", bufs=2)
                        nc.vector._custom_dve(
                            mmd_op, out=scr[:, 0:nfree], in0=cur[:, 0:nfree],
                            accum_out=rid[:, 1:2])

            for u in range(NUNIT):
                nc.sync.dma_start(out=rid_d.ap()[u],
                                  in_=riders[nrep - 1][u][:])

    nc.compile()
    return nc


_PROG = None


def _get_program():
    global _PROG
    if _PROG is None:
        _PROG = _build_program()
    return _PROG


def _prep_inputs(latent):
    X = np.asarray(latent, np.float32)
    X64 = X.astype(np.float64)
    sq = (X64 * X64).sum(1)                      # [N]
    M2 = float(N) * N - N

    def block_d2_sum(lo, hi):
        n = hi - lo
        sv = X64[lo:hi].sum(0)
        return 2.0 * (n * sq[lo:hi].sum()) - 2.0 * (sv @ sv)

    S_src = block_d2_sum(0, HALF)
    S_tgt = block_d2_sum(HALF, N)
    sv_all = X64.sum(0)
    S_full = 2.0 * (N * sq.sum()) - 2.0 * (sv_all @ sv_all)

    bw_xx = S_src / M2           # already includes /mul^(num//2) (see notes)
    bw_yy = S_tgt / M2
    bw_xy = (S_full / M2) / 4.0

    in_maps = []
    for core in range(NCORES):
        lc = _local_cols(core)
        xf = X[lc].T.reshape(4, 128, LC)
        xth = np.ascontiguousarray(xf).astype(ml_dtypes.bfloat16)
        sql = sq[lc]
        v = -0.5 * sql
        hi = np.asarray(v, ml_dtypes.bfloat16).astype(np.float64)
        lo = (v - hi).astype(np.float32)
        hi = hi.astype(np.float32)
        ones = np.ones_like(hi)
        aug = np.zeros((4, AUG_W), ml_dtypes.bfloat16)
        aug[0, :LC] = hi
        aug[1, :LC] = lo
        aug[2, :LC] = ones
        aug[3, :LC] = ones
        # lhsT sections: [1, 1, hi_row, lo_row] for own rows and slotL rows
        for sec, row0 in ((LC, OWN_OFF), (LC + 512, SL_OFF)):
            aug[0, sec:sec + 512] = 1.0
            aug[1, sec:sec + 512] = 1.0
            aug[2, sec:sec + 512] = hi[row0:row0 + 512]
            aug[3, sec:sec + 512] = lo[row0:row0 + 512]

        bw_own = bw_xx if core < 4 else bw_yy
        scales = np.zeros((128, 2), np.float32)
        scales[:, 0] = 1.0 / (8.0 * bw_own)
        scales[:, 1] = 1.0 / (8.0 * bw_xy)
        in_maps.append({"xth": xth, "aug": aug, "scales": scales})
    return in_maps


def _postprocess(results):
    S_own = np.zeros(NCORES)
    S_xy = np.zeros(NCORES)
    for core in range(NCORES):
        r = results[core]["riders"].astype(np.float64)  # [NUNIT,128,RID_W]
        for u, (off, w, l, al, wt, chain) in enumerate(UNITS):
            val = wt * r[u].sum()
            if chain == "own":
                S_own[core] += val
            else:
                S_xy[core] += val
    xx = S_own[:4].sum() / (HALF * HALF)
    yy = S_own[4:].sum() / (HALF * HALF)
    xy = S_xy.sum() / (float(N) * N)
    return np.float32(xx + yy - 2.0 * xy)


def _run(inputs, trace=False, **kw):
    from concourse.bass_utils import run_bass_kernel_spmd
    nc = _get_program()
    in_maps = _prep_inputs(inputs["latent"])
    res = run_bass_kernel_spmd(nc, in_maps, list(range(NCORES)),
                               trace=trace, **kw)
    return _postprocess(res.results), res


def kernel(**inputs):
    out, _ = _run(inputs, trace=False)
    return out


if __name__ == "__main__":
    rng = np.random.default_rng(0)
    lat = rng.standard_normal((N, D)).astype(np.float32)
    print(kernel(latent=lat,
                 domain=np.concatenate([np.zeros(HALF, np.int32),
                                        np.ones(HALF, np.int32)])))


# revision 36
# speedup vs baseline: 1.1398x; 1.1398x over previous
"""MMD loss kernel for Trainium2 (8 NeuronCores, Bass/Tile).

reference math:
  src = X[:2048], tgt = X[2048:],  D=512
  xx = mean over [4096,4096] of sum_k exp(-d2_dup(src,src)/(bw_xx*2^k))
  (dup matrix mean == mean over the 2048^2 block), similarly yy, and
  xy uses the full 4096^2 matrix of X.
  bw for (a,b) = sum(d2([a;b]))/(m^2-m) / mul^(num//2),  mul=2, num=5.

Strategy:
  - bandwidth sums have a closed form: sum_block d2 = 2n*sum(sq) - 2|sum x|^2
    -> computed host-side in fp64, passed to the device as runtime
    activation *scales* (per-partition AP), so no first pass over d2.
  - pairwise tile: PSUM M = G - sq_i/2 - sq_j/2 = -d2/2 via an augmented
    matmul: K=512 data in single-pass bf16 + K=4 aug rows with bf16 hi/lo
    split of -sq/2 (kept high-precision so only the x.y cross term is
    bf16-rounded; full-pipeline rel err ~1.2e-4).
  - 5-kernel sum: u = exp(scale*M) with scale = 1/(8*bw_base) on ACT
    (accum_out rider = sum u), then ONE fused custom-DVE op computes
    u^2+u^4+u^8+u^16 elementwise (8 ALU stages) with accum_out rider =
    its row sum. The host only needs the total of the 5 kernel sums, so
    two riders per chain suffice.
  - symmetry: the distance matrix is symmetric. Own-half coverage is
    minimal: each core computes its diagonal block, one adjacent pair
    block (w2), and HALF (256 cols) of a skip pair (w2) — the two halves
    of each skip pair are computed from the SAME lhsT side by two cores
    (both hold those rows in a dedicated slotL), so the union tiles the
    pair block exactly. The diagonal block additionally exploits its own
    symmetry at 256-col granularity (S10 = S01^T): only S00+S11 (w1) and
    S01 (w2) are computed, streaming 1536 of 2048 cols. Cross src/tgt
    blocks are covered once with weight 2 across the 8 cores. Every core
    runs the SAME program on a per-core permuted column layout.
  - timing contract: only the final rep's riders are DMA'd out, so the
    output size is independent of REPEAT and the wall-delta between
    REPEAT variants isolates on-device body time.
"""

import sys

sys.path.insert(0, "/opt/trn_rl_repo")

import numpy as np
import ml_dtypes

N, D, HALF, BLK = 4096, 512, 2048, 512
NCORES = 8
NSTRIP = 4          # 4 strips of 128 rows per core
RID_W = 2           # rider slots per unit: [sum u, sum u^2+u^4+u^8+u^16]

# Local column layout (units of X-row indices), per core:
#   own   [0:512)      core's own row group g = 4*half + k0
#   adj   [512:1024)   group (k0+1)%4 of the same half      -> pair w2
#   slotL [1024:1536)  lhsT rows for the skip pair: group k0%2
#   slotR [1536:1792)  rhs cols for the skip pair: 256-col half of
#                      group (k0%2)+2  (first half if k0<2 else second)
#   cross [1792:2816)  two 512-col groups of the other half -> w2, xy only
# Own-half coverage proof: diag (g,g) w1 by each core; adjacent pairs
# (k,k+1 mod 4) w2 once each; skip pairs (0,2),(1,3) w2 split into two
# 256-col halves, both computed with the SAME lhsT side (cores k and k+2
# hold those rows in slotL), so the union tiles the pair block exactly.
LC = 2816
AUG_W = LC + 1024   # + [1,1,hi,lo] lhsT sections for own and slotL
OWN_OFF, ADJ_OFF, SL_OFF, SR_OFF = 0, 512, 1024, 1536
CR0_OFF, CR1_OFF = 1792, 2304

# units: (kind, weight, chain). The diag block exploits its own symmetry at
# 256-col granularity: S10 = S01^T, so only S00+S11 (w1) and S01 (w2) are
# computed — 1536 of 2048 cols streamed. The short skip-pair instance goes
# LAST: in 3rd position its small matmul burst starves the PSUM
# double-buffer pipeline (sim: 21.3us vs 19.2us/rep).
def _units():
    us = []
    for kind, wt in (("diag1", 1), ("diag2", 2)):
        us.append((kind, wt, "own"))
        us.append((kind, wt, "xy"))
    us.append(("adj", 2, "own"))
    us.append(("adj", 2, "xy"))
    us.append(("c0", 2, "xy"))
    us.append(("c1", 2, "xy"))
    us.append(("skip", 2, "own"))
    us.append(("skip", 2, "xy"))
    return us


UNITS = _units()
NUNIT = len(UNITS)  # 8

# NOTE: fp8-e4m3 DoubleRow Gram matmuls (K=256/instruction) were tried and
# measured SLOWER on real HW than plain bf16 (26.1us vs 24.1us body) despite
# the cost model predicting a win — DR LDWEIGHTS overhead isn't hidden here.
MM_DT = "bfloat16"
U_DT = "float32"    # dtype of the exp output / fused-op scratch tiles


REPEAT = 1


_MMD_OP = None


def _get_mmd_op():
    """Fused DVE op: out = u^2+u^4+u^8+u^16, accum_out = row-sum(out).

    Registered once into dve_ops.OPS (the sanctioned custom-DVE extension
    point; the uop table is emitted per-NEFF at compile time)."""
    global _MMD_OP
    if _MMD_OP is not None:
        return _MMD_OP
    from concourse import dve_ops
    from concourse.dve_spec import Spec, Src0, sq, lower
    from concourse.dve_uop import AluOp, DveOpSpec

    name = "MMD_POW_SUM"
    for op in dve_ops.OPS:
        if op.name == name:
            _MMD_OP = op
            return op

    a = sq(Src0)
    b = sq(a)
    c = sq(b)
    d = sq(c)

    def _ref(in0, in1, c0, c1, c2):
        x = in0.astype(np.float32)
        aa = x * x
        bb = aa * aa
        cc = bb * bb
        dd = cc * cc
        body = (aa + bb) + (cc + dd)
        return body, body.reshape(body.shape[0], -1).sum(
            axis=-1, keepdims=True)

    spec = Spec(body=(a + b) + (c + d), accum=AluOp.ADD, reference=_ref)
    row = max(dve_ops._SUB_OPCODE_FOR_NAME.values()) + 1
    assert row < 0x20, "custom-DVE opcode rows exhausted"
    shas = {}
    for ver in ("v3", "v4"):
        uops = lower(spec, ver=ver)
        shas[ver] = DveOpSpec(name=name, opcode=row, uops=uops,
                              rd1_en=False).sha(ver)
    op = dve_ops.DveOp(name, spec, subdim=False, uops_sha=shas)
    dve_ops.OPS.append(op)
    dve_ops._SUB_OPCODE_FOR_NAME[name] = row
    dve_ops.CUSTOM_DVE_SPECS[name] = spec
    _MMD_OP = op
    return op


def _local_cols(core):
    half, k = core // 4, core % 4
    own_base, other_base = half * HALF, (1 - half) * HALF
    own = own_base + 512 * k + np.arange(512)
    adj = own_base + 512 * ((k + 1) % 4) + np.arange(512)
    sl = own_base + 512 * (k % 2) + np.arange(512)
    sr_g = (k % 2) + 2
    sr_base = own_base + 512 * sr_g + (0 if k < 2 else 256)
    sr = sr_base + np.arange(256)
    if half == 0:
        cross = [0, 1] if k % 2 == 0 else [2, 3]
    else:
        cross = [1, 3] if k < 2 else [0, 2]
    cr = [other_base + 512 * b + np.arange(512) for b in cross]
    return np.concatenate([own, adj, sl, sr] + cr)


def _build_program():
    import concourse.bacc as bacc
    import concourse.mybir as mybir
    import concourse.tile as tile

    f32 = mybir.dt.float32
    mm_dt = getattr(mybir.dt, MM_DT)
    u_dt = getattr(mybir.dt, U_DT)
    mmd_op = _get_mmd_op()

    nc = bacc.Bacc("TRN2", target_bir_lowering=False, debug=False,
                   num_devices=NCORES)
    xth_d = nc.dram_tensor("xth", [4, 128, LC], mm_dt, kind="ExternalInput")
    aug_d = nc.dram_tensor("aug", [4, AUG_W], mm_dt, kind="ExternalInput")
    sc_d = nc.dram_tensor("scales", [128, 2], f32, kind="ExternalInput")
    nrep = globals().get("REPEAT", 1)
    # Output only the final rep's riders: keeps the output tensor size
    # independent of REPEAT so the wall-delta timing isolates device time
    # (otherwise the tunnel transfer of the extra output dominates).
    rid_d = nc.dram_tensor("riders", [NUNIT, 128, RID_W], f32,
                           kind="ExternalOutput")

    a_own, a_sl = LC, LC + 512
    # instance plans: (mms, chains); mms = [(s, ps_off, w, rhs_off, lhsT_off,
    # aug_lhsT_off)], chains = [(unit, exp_off, exp_w)]
    def plan(kind):
        if kind == "diag":
            mms = [(0, 0, 256, OWN_OFF, OWN_OFF, a_own),
                   (0, 1024, 256, OWN_OFF + 256, OWN_OFF, a_own),
                   (1, 256, 256, OWN_OFF, OWN_OFF, a_own),
                   (1, 1280, 256, OWN_OFF + 256, OWN_OFF, a_own),
                   (2, 512, 256, OWN_OFF + 256, OWN_OFF, a_own),
                   (3, 768, 256, OWN_OFF + 256, OWN_OFF, a_own)]
            chains = [(0, 0, 1024), (1, 0, 1024),      # diag1 own/xy (w1)
                      (2, 1024, 512), (3, 1024, 512)]  # diag2 own/xy (w2)
        elif kind in ("adj", "c0", "c1"):
            off = {"adj": ADJ_OFF, "c0": CR0_OFF, "c1": CR1_OFF}[kind]
            mms = [(s, 512 * s, 512, off, OWN_OFF, a_own) for s in range(4)]
            chains = {"adj": [(4, 0, 2048), (5, 0, 2048)],
                      "c0": [(6, 0, 2048)],
                      "c1": [(7, 0, 2048)]}[kind]
        else:  # skip
            mms = [(s, 256 * s, 256, SR_OFF, SL_OFF, a_sl) for s in range(4)]
            chains = [(8, 0, 1024), (9, 0, 1024)]
        return mms, chains

    with tile.TileContext(nc) as tc:
        with (
            tc.tile_pool(name="xtp", bufs=1) as xtp,
            tc.tile_pool(name="augp", bufs=1) as augp,
            tc.tile_pool(name="scp", bufs=1) as scp,
            tc.tile_pool(name="ridp", bufs=1) as ridp,
            tc.tile_pool(name="psp", bufs=8, space="PSUM") as psp,
            tc.tile_pool(name="up", bufs=4) as up,
        ):
            xth = [xtp.tile([128, LC], mm_dt, tag=f"xth{k}",
                            name=f"xth{k}") for k in range(4)]
            for k in range(4):
                nc.sync.dma_start(out=xth[k][:], in_=xth_d.ap()[k])
            aug = augp.tile([4, AUG_W], mm_dt, tag="aug", name="aug")
            sc = scp.tile([128, 2], f32, tag="sc", name="sc")
            nc.sync.dma_start(out=aug[:], in_=aug_d.ap())
            nc.sync.dma_start(out=sc[:], in_=sc_d.ap())

            riders = [[ridp.tile([128, RID_W], f32, tag=f"rid{u}_{rp}",
                                 name=f"rid{u}_{rp}") for u in range(NUNIT)]
                      for rp in range(nrep)]

            for rep in range(nrep):
                for kind in ("diag", "adj", "c0", "c1", "skip"):
                    mms, chains = plan(kind)
                    ps = psp.tile([128, 2048], f32, tag="ps", name="ps",
                                  bufs=2)
                    for s, po, w, off, l, al in mms:
                        pss = ps[:, po:po + w]
                        for k in range(4):
                            lh = xth[k][:, l + 128 * s:l + 128 * s + 128]
                            rh = xth[k][:, off:off + w]
                            nc.tensor.matmul(out=pss, lhsT=lh, rhs=rh,
                                             start=(k == 0), stop=False)
                        nc.tensor.matmul(
                            out=pss,
                            lhsT=aug[:, al + 128 * s:al + 128 * s + 128],
                            rhs=aug[:, off:off + w],
                            start=False, stop=True)

                    for u, eo, ew in chains:
                        chain = UNITS[u][2]
                        rid = riders[rep][u]
                        sci = 0 if chain == "own" else 1
                        cur = up.tile([128, 2048], u_dt, tag="u", name="u",
                                      bufs=3)
                        nc.scalar.activation(
                            out=cur[:, 0:ew], in_=ps[:, eo:eo + ew],
                            func=mybir.ActivationFunctionType.Exp,
                            scale=sc[:, sci:sci + 1],
                            accum_out=rid[:, 0:1])
                        scr = up.tile([128, 2048], u_dt, tag="usq",
                                      name="usq", bufs=2)
                        nc.vector._custom_dve(
                            mmd_op, out=scr[:, 0:ew], in0=cur[:, 0:ew],
                            accum_out=rid[:, 1:2])

            for u in range(NUNIT):
                nc.sync.dma_start(out=rid_d.ap()[u],
                                  in_=riders[nrep - 1][u][:])

    nc.compile()
    return nc


_PROG = None


def _get_program():
    global _PROG
    if _PROG is None:
        _PROG = _build_program()
    return _PROG


def _prep_inputs(latent):
    X = np.asarray(latent, np.float32)
    X64 = X.astype(np.float64)
    sq = (X64 * X64).sum(1)                      # [N]
    M2 = float(N) * N - N

    def block_d2_sum(lo, hi):
        n = hi - lo
        sv = X64[lo:hi].sum(0)
        return 2.0 * (n * sq[lo:hi].sum()) - 2.0 * (sv @ sv)

    S_src = block_d2_sum(0, HALF)
    S_tgt = block_d2_sum(HALF, N)
    sv_all = X64.sum(0)
    S_full = 2.0 * (N * sq.sum()) - 2.0 * (sv_all @ sv_all)

    bw_xx = S_src / M2           # already includes /mul^(num//2) (see notes)
    bw_yy = S_tgt / M2
    bw_xy = (S_full / M2) / 4.0

    in_maps = []
    for core in range(NCORES):
        lc = _local_cols(core)
        xf = X[lc].T.reshape(4, 128, LC)
        xth = np.ascontiguousarray(xf).astype(ml_dtypes.bfloat16)
        sql = sq[lc]
        v = -0.5 * sql
        hi = np.asarray(v, ml_dtypes.bfloat16).astype(np.float64)
        lo = (v - hi).astype(np.float32)
        hi = hi.astype(np.float32)
        ones = np.ones_like(hi)
        aug = np.zeros((4, AUG_W), ml_dtypes.bfloat16)
        aug[0, :LC] = hi
        aug[1, :LC] = lo
        aug[2, :LC] = ones
        aug[3, :LC] = ones
        # lhsT sections: [1, 1, hi_row, lo_row] for own rows and slotL rows
        for sec, row0 in ((LC, OWN_OFF), (LC + 512, SL_OFF)):
            aug[0, sec:sec + 512] = 1.0
            aug[1, sec:sec + 512] = 1.0
            aug[2, sec:sec + 512] = hi[row0:row0 + 512]
            aug[3, sec:sec + 512] = lo[row0:row0 + 512]

        bw_own = bw_xx if core < 4 else bw_yy
        scales = np.zeros((128, 2), np.float32)
        scales[:, 0] = 1.0 / (8.0 * bw_own)
        scales[:, 1] = 1.0 / (8.0 * bw_xy)
        in_maps.append({"xth": xth, "aug": aug, "scales": scales})
    return in_maps


def _postprocess(results):
    S_own = np.zeros(NCORES)
    S_xy = np.zeros(NCORES)
    for core in range(NCORES):
        r = results[core]["riders"].astype(np.float64)  # [NUNIT,128,RID_W]
        for u, (kind, wt, chain) in enumerate(UNITS):
            val = wt * r[u].sum()
            if chain == "own":
                S_own[core] += val
            else:
                S_xy[core] += val
    xx = S_own[:4].sum() / (HALF * HALF)
    yy = S_own[4:].sum() / (HALF * HALF)
    xy = S_xy.sum() / (float(N) * N)
    return np.float32(xx + yy - 2.0 * xy)


def _run(inputs, trace=False, **kw):
    from concourse.bass_utils import run_bass_kernel_spmd
    nc = _get_program()
    in_maps = _prep_inputs(inputs["latent"])
    res = run_bass_kernel_spmd(nc, in_maps, list(range(NCORES)),
                               trace=trace, **kw)
    return _postprocess(res.results), res


def kernel(**inputs):
    out, _ = _run(inputs, trace=False)
    return out


if __name__ == "__main__":
    rng = np.random.default_rng(0)
    lat = rng.standard_normal((N, D)).astype(np.float32)
    print(kernel(latent=lat,
                 domain=np.concatenate([np.zeros(HALF, np.int32),
                                        np.ones(HALF, np.int32)])))


# revision 40
# speedup vs baseline: 1.2042x; 1.0565x over previous
"""MMD loss kernel for Trainium2 (8 NeuronCores, Bass/Tile).

reference math:
  src = X[:2048], tgt = X[2048:],  D=512
  xx = mean over [4096,4096] of sum_k exp(-d2_dup(src,src)/(bw_xx*2^k))
  (dup matrix mean == mean over the 2048^2 block), similarly yy, and
  xy uses the full 4096^2 matrix of X.
  bw for (a,b) = sum(d2([a;b]))/(m^2-m) / mul^(num//2),  mul=2, num=5.

Strategy:
  - bandwidth sums have a closed form: sum_block d2 = 2n*sum(sq) - 2|sum x|^2
    -> computed host-side in fp64, passed to the device as runtime
    activation *scales* (per-partition AP), so no first pass over d2.
  - pairwise tile: PSUM M = G - sq_i/2 - sq_j/2 = -d2/2 via an augmented
    matmul: K=512 data in single-pass bf16 + K=4 aug rows with bf16 hi/lo
    split of -sq/2 (kept high-precision so only the x.y cross term is
    bf16-rounded; full-pipeline rel err ~1.2e-4).
  - 5-kernel sum: u = exp(scale*M) with scale = 1/(8*bw_base) on ACT
    (accum_out rider = sum u), then ONE fused custom-DVE op computes
    u^2+u^4+u^8+u^16 elementwise (8 ALU stages) with accum_out rider =
    its row sum. The host only needs the total of the 5 kernel sums, so
    two riders per chain suffice.
  - symmetry: the distance matrix is symmetric. Own-half coverage is
    minimal: each core computes its diagonal block, one adjacent pair
    block (w2), and HALF (256 cols) of a skip pair (w2) — the two halves
    of each skip pair are computed from the SAME lhsT side by two cores
    (both hold those rows in a dedicated slotL), so the union tiles the
    pair block exactly. The diagonal block additionally exploits its own
    symmetry at 256-col granularity (S10 = S01^T): only S00+S11 (w1) and
    S01 (w2) are computed, streaming 1536 of 2048 cols. Cross src/tgt
    blocks are covered once with weight 2 across the 8 cores. Every core
    runs the SAME program on a per-core permuted column layout.
  - timing contract: only the final rep's riders are DMA'd out, so the
    output size is independent of REPEAT and the wall-delta between
    REPEAT variants isolates on-device body time.
"""

import sys

sys.path.insert(0, "/opt/trn_rl_repo")

import numpy as np
import ml_dtypes

N, D, HALF, BLK = 4096, 512, 2048, 512
NCORES = 8
NSTRIP = 4          # 4 strips of 128 rows per core
RID_W = 2           # rider slots per unit: [sum u, sum u^2+u^4+u^8+u^16]

# Local column layout (units of X-row indices), per core:
#   own   [0:512)      core's own row group g = 4*half + k0
#   adj   [512:1024)   group (k0+1)%4 of the same half      -> pair w2
#   slotL [1024:1536)  lhsT rows for the skip pair: group k0%2
#   slotR [1536:1792)  rhs cols for the skip pair: 256-col half of
#                      group (k0%2)+2  (first half if k0<2 else second)
#   cross [1792:2816)  two 512-col groups of the other half -> w2, xy only
# Own-half coverage proof: diag (g,g) w1 by each core; adjacent pairs
# (k,k+1 mod 4) w2 once each; skip pairs (0,2),(1,3) w2 split into two
# 256-col halves, both computed with the SAME lhsT side (cores k and k+2
# hold those rows in slotL), so the union tiles the pair block exactly.
LC = 2816
AUG_W = LC + 1024   # + [1,1,hi,lo] lhsT sections for own and slotL
OWN_OFF, ADJ_OFF, SL_OFF, SR_OFF = 0, 512, 1024, 1536
CR0_OFF, CR1_OFF = 1792, 2304

# units: (kind, weight, chain). The diag block exploits its own symmetry at
# 256-col granularity: S10 = S01^T, so only S00+S11 (w1) and S01 (w2) are
# computed — 1536 of 2048 cols streamed. The short skip-pair instance goes
# LAST: in 3rd position its small matmul burst starves the PSUM
# double-buffer pipeline (sim: 21.3us vs 19.2us/rep).
def _units():
    us = []
    for kind, wt in (("diag1", 1), ("diag2", 2)):
        us.append((kind, wt, "own"))
        us.append((kind, wt, "xy"))
    us.append(("adj", 2, "own"))
    us.append(("adj", 2, "xy"))
    us.append(("c0", 2, "xy"))
    us.append(("c1", 2, "xy"))
    us.append(("skip", 2, "own"))
    us.append(("skip", 2, "xy"))
    return us


UNITS = _units()
NUNIT = len(UNITS)  # 8

# NOTE: fp8-e4m3 DoubleRow Gram matmuls (K=256/instruction) were tried and
# measured SLOWER on real HW than plain bf16 (26.1us vs 24.1us body) despite
# the cost model predicting a win — DR LDWEIGHTS overhead isn't hidden here.
MM_DT = "bfloat16"
U_DT = "float32"    # dtype of the exp output / fused-op scratch tiles


REPEAT = 1


_MMD_OP = None


def _get_mmd_op():
    """Fused DVE op: out = u^2+u^4+u^8+u^16, accum_out = row-sum(out).

    Registered once into dve_ops.OPS (the sanctioned custom-DVE extension
    point; the uop table is emitted per-NEFF at compile time)."""
    global _MMD_OP
    if _MMD_OP is not None:
        return _MMD_OP
    from concourse import dve_ops
    from concourse.dve_spec import Spec, Src0, sq, lower
    from concourse.dve_uop import AluOp, DveOpSpec

    name = "MMD_POW_SUM"
    for op in dve_ops.OPS:
        if op.name == name:
            _MMD_OP = op
            return op

    a = sq(Src0)
    b = sq(a)
    c = sq(b)
    d = sq(c)

    def _ref(in0, in1, c0, c1, c2):
        x = in0.astype(np.float32)
        aa = x * x
        bb = aa * aa
        cc = bb * bb
        dd = cc * cc
        body = (aa + bb) + (cc + dd)
        return body, body.reshape(body.shape[0], -1).sum(
            axis=-1, keepdims=True)

    spec = Spec(body=(a + b) + (c + d), accum=AluOp.ADD, reference=_ref)
    row = max(dve_ops._SUB_OPCODE_FOR_NAME.values()) + 1
    assert row < 0x20, "custom-DVE opcode rows exhausted"
    shas = {}
    for ver in ("v3", "v4"):
        uops = lower(spec, ver=ver)
        shas[ver] = DveOpSpec(name=name, opcode=row, uops=uops,
                              rd1_en=False).sha(ver)
    op = dve_ops.DveOp(name, spec, subdim=False, uops_sha=shas)
    dve_ops.OPS.append(op)
    dve_ops._SUB_OPCODE_FOR_NAME[name] = row
    dve_ops.CUSTOM_DVE_SPECS[name] = spec
    _MMD_OP = op
    return op


def _local_cols(core):
    half, k = core // 4, core % 4
    own_base, other_base = half * HALF, (1 - half) * HALF
    own = own_base + 512 * k + np.arange(512)
    adj = own_base + 512 * ((k + 1) % 4) + np.arange(512)
    sl = own_base + 512 * (k % 2) + np.arange(512)
    sr_g = (k % 2) + 2
    sr_base = own_base + 512 * sr_g + (0 if k < 2 else 256)
    sr = sr_base + np.arange(256)
    if half == 0:
        cross = [0, 1] if k % 2 == 0 else [2, 3]
    else:
        cross = [1, 3] if k < 2 else [0, 2]
    cr = [other_base + 512 * b + np.arange(512) for b in cross]
    return np.concatenate([own, adj, sl, sr] + cr)


def _build_program():
    import concourse.bacc as bacc
    import concourse.mybir as mybir
    import concourse.tile as tile

    f32 = mybir.dt.float32
    mm_dt = getattr(mybir.dt, MM_DT)
    u_dt = getattr(mybir.dt, U_DT)
    mmd_op = _get_mmd_op()

    nc = bacc.Bacc("TRN2", target_bir_lowering=False, debug=False,
                   num_devices=NCORES)
    xth_d = nc.dram_tensor("xth", [4, 128, LC], mm_dt, kind="ExternalInput")
    aug_d = nc.dram_tensor("aug", [4, AUG_W], mm_dt, kind="ExternalInput")
    sc_d = nc.dram_tensor("scales", [128, 2], f32, kind="ExternalInput")
    nrep = globals().get("REPEAT", 1)
    # Output only the final rep's riders: keeps the output tensor size
    # independent of REPEAT so the wall-delta timing isolates device time
    # (otherwise the tunnel transfer of the extra output dominates).
    rid_d = nc.dram_tensor("riders", [NUNIT, 128, RID_W], f32,
                           kind="ExternalOutput")

    a_own, a_sl = LC, LC + 512
    # instance plans: (mms, chains); mms = [(s, ps_off, w, rhs_off, lhsT_off,
    # aug_lhsT_off)], chains = [(unit, exp_off, exp_w)]
    def plan(kind):
        if kind == "diag":
            mms = [(0, 0, 256, OWN_OFF, OWN_OFF, a_own),
                   (0, 1024, 256, OWN_OFF + 256, OWN_OFF, a_own),
                   (1, 256, 256, OWN_OFF, OWN_OFF, a_own),
                   (1, 1280, 256, OWN_OFF + 256, OWN_OFF, a_own),
                   (2, 512, 256, OWN_OFF + 256, OWN_OFF, a_own),
                   (3, 768, 256, OWN_OFF + 256, OWN_OFF, a_own)]
            chains = [(0, 0, 1024), (1, 0, 1024),      # diag1 own/xy (w1)
                      (2, 1024, 512), (3, 1024, 512)]  # diag2 own/xy (w2)
        elif kind in ("adj", "c0", "c1"):
            off = {"adj": ADJ_OFF, "c0": CR0_OFF, "c1": CR1_OFF}[kind]
            mms = [(s, 512 * s, 512, off, OWN_OFF, a_own) for s in range(4)]
            chains = {"adj": [(4, 0, 2048), (5, 0, 2048)],
                      "c0": [(6, 0, 2048)],
                      "c1": [(7, 0, 2048)]}[kind]
        else:  # skip
            mms = [(s, 256 * s, 256, SR_OFF, SL_OFF, a_sl) for s in range(4)]
            chains = [(8, 0, 1024), (9, 0, 1024)]
        return mms, chains

    with tile.TileContext(nc) as tc:
        with (
            tc.tile_pool(name="xtp", bufs=1) as xtp,
            tc.tile_pool(name="augp", bufs=1) as augp,
            tc.tile_pool(name="scp", bufs=1) as scp,
            tc.tile_pool(name="ridp", bufs=1) as ridp,
            tc.tile_pool(name="psp", bufs=8, space="PSUM") as psp,
            tc.tile_pool(name="up", bufs=4) as up,
        ):
            xth = [xtp.tile([128, LC], mm_dt, tag=f"xth{k}",
                            name=f"xth{k}") for k in range(4)]
            for k in range(4):
                nc.sync.dma_start(out=xth[k][:], in_=xth_d.ap()[k])
            aug = augp.tile([4, AUG_W], mm_dt, tag="aug", name="aug")
            sc = scp.tile([128, 2], f32, tag="sc", name="sc")
            nc.sync.dma_start(out=aug[:], in_=aug_d.ap())
            nc.sync.dma_start(out=sc[:], in_=sc_d.ap())

            riders = [[ridp.tile([128, RID_W], f32, tag=f"rid{u}_{rp}",
                                 name=f"rid{u}_{rp}") for u in range(NUNIT)]
                      for rp in range(nrep)]

            for rep in range(nrep):
                for kind in ("diag", "adj", "c0", "c1", "skip"):
                    mms, chains = plan(kind)
                    ps = psp.tile([128, 2048], f32, tag="ps", name="ps",
                                  bufs=2)
                    for s, po, w, off, l, al in mms:
                        pss = ps[:, po:po + w]
                        for k in range(4):
                            lh = xth[k][:, l + 128 * s:l + 128 * s + 128]
                            rh = xth[k][:, off:off + w]
                            nc.tensor.matmul(out=pss, lhsT=lh, rhs=rh,
                                             start=(k == 0), stop=False)
                        nc.tensor.matmul(
                            out=pss,
                            lhsT=aug[:, al + 128 * s:al + 128 * s + 128],
                            rhs=aug[:, off:off + w],
                            start=False, stop=True)

                    for u, eo, ew in chains:
                        chain = UNITS[u][2]
                        rid = riders[rep][u]
                        sci = 0 if chain == "own" else 1
                        cur = up.tile([128, 2048], u_dt, tag="u", name="u",
                                      bufs=3)
                        nc.scalar.activation(
                            out=cur[:, 0:ew], in_=ps[:, eo:eo + ew],
                            func=mybir.ActivationFunctionType.Exp,
                            scale=sc[:, sci:sci + 1],
                            accum_out=rid[:, 0:1])
                        scr = up.tile([128, 2048], u_dt, tag="usq",
                                      name="usq", bufs=2)
                        nc.vector._custom_dve(
                            mmd_op, out=scr[:, 0:ew], in0=cur[:, 0:ew],
                            accum_out=rid[:, 1:2])

            for u in range(NUNIT):
                nc.sync.dma_start(out=rid_d.ap()[u],
                                  in_=riders[nrep - 1][u][:])

    nc.compile()
    return nc


_PROG = None


def _get_program():
    global _PROG
    if _PROG is None:
        _PROG = _build_program()
    return _PROG


def _prep_inputs(latent):
    X = np.asarray(latent, np.float32)
    X64 = X.astype(np.float64)
    sq = (X64 * X64).sum(1)                      # [N]
    M2 = float(N) * N - N

    def block_d2_sum(lo, hi):
        n = hi - lo
        sv = X64[lo:hi].sum(0)
        return 2.0 * (n * sq[lo:hi].sum()) - 2.0 * (sv @ sv)

    S_src = block_d2_sum(0, HALF)
    S_tgt = block_d2_sum(HALF, N)
    sv_all = X64.sum(0)
    S_full = 2.0 * (N * sq.sum()) - 2.0 * (sv_all @ sv_all)

    bw_xx = S_src / M2           # already includes /mul^(num//2) (see notes)
    bw_yy = S_tgt / M2
    bw_xy = (S_full / M2) / 4.0

    in_maps = []
    for core in range(NCORES):
        lc = _local_cols(core)
        xf = X[lc].T.reshape(4, 128, LC)
        xth = np.ascontiguousarray(xf).astype(ml_dtypes.bfloat16)
        sql = sq[lc]
        v = -0.5 * sql
        hi = np.asarray(v, ml_dtypes.bfloat16).astype(np.float64)
        lo = (v - hi).astype(np.float32)
        hi = hi.astype(np.float32)
        ones = np.ones_like(hi)
        aug = np.zeros((4, AUG_W), ml_dtypes.bfloat16)
        aug[0, :LC] = hi
        aug[1, :LC] = lo
        aug[2, :LC] = ones
        aug[3, :LC] = ones
        # lhsT sections: [1, 1, hi_row, lo_row] for own rows and slotL rows
        for sec, row0 in ((LC, OWN_OFF), (LC + 512, SL_OFF)):
            aug[0, sec:sec + 512] = 1.0
            aug[1, sec:sec + 512] = 1.0
            aug[2, sec:sec + 512] = hi[row0:row0 + 512]
            aug[3, sec:sec + 512] = lo[row0:row0 + 512]

        bw_own = bw_xx if core < 4 else bw_yy
        scales = np.zeros((128, 2), np.float32)
        scales[:, 0] = 1.0 / (8.0 * bw_own)
        scales[:, 1] = 1.0 / (8.0 * bw_xy)
        in_maps.append({"xth": xth, "aug": aug, "scales": scales})
    return in_maps


def _postprocess(results):
    S_own = np.zeros(NCORES)
    S_xy = np.zeros(NCORES)
    for core in range(NCORES):
        r = results[core]["riders"].astype(np.float64)  # [NUNIT,128,RID_W]
        for u, (kind, wt, chain) in enumerate(UNITS):
            val = wt * r[u].sum()
            if chain == "own":
                S_own[core] += val
            else:
                S_xy[core] += val
    xx = S_own[:4].sum() / (HALF * HALF)
    yy = S_own[4:].sum() / (HALF * HALF)
    xy = S_xy.sum() / (float(N) * N)
    return np.float32(xx + yy - 2.0 * xy)


def _run(inputs, trace=False, **kw):
    from concourse.bass_utils import run_bass_kernel_spmd
    nc = _get_program()
    in_maps = _prep_inputs(inputs["latent"])
    res = run_bass_kernel_spmd(nc, in_maps, list(range(NCORES)),
                               trace=trace, **kw)
    return _postprocess(res.results), res


def kernel(**inputs):
    out, _ = _run(inputs, trace=False)
    return out


if __name__ == "__main__":
    rng = np.random.default_rng(0)
    lat = rng.standard_normal((N, D)).astype(np.float32)
    print(kernel(latent=lat,
                 domain=np.concatenate([np.zeros(HALF, np.int32),
                                        np.ones(HALF, np.int32)])))
